# revision 32
# baseline (speedup 1.0000x reference)
"""Octahedral SHT on 8 NeuronCores (Bass/Tile), quarter-folded fp16 design.

Strategy: shard the 192 latitude rings across 8 cores (24 each). The ragged
per-ring DFT is quarter-folded on the host using the cosine/sine symmetries
j<->n-j and j<->n/2-j: the folded x vectors (we/wo/ze/zo, one per m-parity x
re/im quadrant) have n/4+-1 rows, so every ring fits a single K<=101 matmul
chunk and the E matrix shrinks 4x. Tolerance is 2e-2, so all operands are
plain fp16 (measured pipeline error ~4.7e-4) - no hi/lo splitting.

Phase 1 (per ring): 4 MMs (re/im x even/odd m) -> psum [128 m~, 256 re|im],
rows 0:64 = even m, 64:128 = odd m. Evacuate f32->f16 (ACT/DVE alternate),
bounce through DRAM to transpose ring-index onto partitions.

Phase 2 (per m): out[bev, l] = G'[r, bev].T @ pw[r, l] with exact triangular
l-range (coeffs vanish for l < m). m's are processed in pairs (m~, 127-m~)
so each psum bank holds exactly 2*l(a)+2*l(b) = 258 f32 columns; one strided
evac per 2-bank tile, fp16 triangular output, host sums 8 partials.

DMA layouts are all contiguous DRAM blocks (>=96KB) with several transfers
in flight per queue: measured ~180GB/s aggregate vs ~20-90GB/s for strided
multi-row patterns. Rings are assigned to (core, slot) by sorted length
rank (slot s = rank 8s..8s+7, one per core) so the per-slot staircase row
count R_S[s] = 101-4s is SPMD-uniform and cuts zero-padding DMA ~45%.
"""
import numpy as np

NLAT, LMAX, MMAX = 192, 128, 128
B, V = 2, 64
BF = B * V            # 128 fused batch (b*64+v)
NCORES = 8
SLOTS = 24            # rings per core
JP = 104              # xef row-pad (>= max folded rows 101)
NPTS = 40320
PWCOLS = 129 * 64     # 8256: pair (a, 127-a) always has l_a + l_b = 129
NTILES = 32           # phase-2 psum tiles (2 m~ pairs each)


def _octa_nlon():
    half = NLAT // 2
    north = np.array([4 * (i + 1) + 16 for i in range(half)], dtype=np.int64)
    return np.concatenate([north, north[::-1]])


def _plan():
    nlon = _octa_nlon()
    order = np.argsort(-nlon, kind="stable")          # ring ids, length desc
    r_s = [int(nlon[order[8 * s]]) // 4 + 1 for s in range(SLOTS)]
    pairs = [(16 * bp + i, 127 - (16 * bp + i))
             for bp in range(4) for i in range(16)]   # m~ pairs, bp-major
    return nlon, order, r_s, pairs


def _true_m(mt):
    return 2 * mt if mt < 64 else 2 * (mt - 64) + 1


def _fold_ring(xr, n):
    """xr [BF, n] f32 -> (we, wo, ze, zo) with q+1, q, q-1, q rows (q=n//4)."""
    h, q = n // 2, n // 4
    u = np.empty((BF, h + 1), np.float32)
    u[:, 0] = xr[:, 0]
    u[:, h] = xr[:, h]
    u[:, 1:h] = xr[:, 1:h] + xr[:, :h:-1]
    v = xr[:, 1:h] - xr[:, :h:-1]                      # j=1..h-1 at col j-1
    jj = np.arange(1, q)
    we = np.empty((BF, q + 1), np.float32)
    we[:, 0] = u[:, 0] + u[:, h]
    we[:, q] = u[:, q]
    we[:, jj] = u[:, jj] + u[:, h - jj]
    wo = np.empty((BF, q), np.float32)
    wo[:, 0] = u[:, 0] - u[:, h]
    wo[:, jj] = u[:, jj] - u[:, h - jj]
    ze = v[:, jj - 1] - v[:, h - jj - 1]               # [BF, q-1]
    zo = np.empty((BF, q), np.float32)
    zo[:, jj - 1] = v[:, jj - 1] + v[:, h - jj - 1]
    zo[:, q - 1] = v[:, q - 1]
    return we, wo, ze, zo


def _build_core_inputs(c, nlon, order, r_s, pairs, offs, x, E_re, E_im, Pw):
    xef = np.zeros((SLOTS, JP, 768), np.float16)
    pw = np.zeros((SLOTS, PWCOLS), np.float16)
    for s in range(SLOTS):
        gid = int(order[8 * s + c])
        n = int(nlon[gid]); q = n // 4; o = int(offs[gid])
        we, wo, ze, zo = _fold_ring(x[:, o:o + n], n)
        xef[s, 0:q + 1, 0:128] = we.T
        xef[s, 0:q,     128:256] = wo.T
        xef[s, 0:q - 1, 256:384] = ze.T
        xef[s, 0:q,     384:512] = zo.T
        xef[s, 0:q + 1, 512:576] = E_re[gid, 0:q + 1, 0::2]
        xef[s, 0:q,     576:640] = E_re[gid, 0:q, 1::2]
        xef[s, 0:q - 1, 640:704] = E_im[gid, 1:q, 0::2]
        xef[s, 0:q,     704:768] = E_im[gid, 1:q + 1, 1::2]
        for p, (a, b) in enumerate(pairs):
            ma, mb = _true_m(a), _true_m(b)
            la = 128 - ma
            pw[s, 129 * p:129 * p + la] = Pw[ma:, ma, gid]
            pw[s, 129 * p + la:129 * (p + 1)] = Pw[mb:, mb, gid]
    return {"xef": xef, "pw": pw}


def _build_bass(r_s, pairs):
    import concourse.bass as bass
    import concourse.mybir as mybir
    from concourse import bacc, tile

    dt = mybir.dt
    nc = bacc.Bacc()

    xef_d = nc.dram_tensor("xef", [SLOTS, JP, 768], dt.float16,
                           kind="ExternalInput")
    pw_d = nc.dram_tensor("pw", [SLOTS, PWCOLS], dt.float16,
                          kind="ExternalInput")
    outp_d = nc.dram_tensor("outp", [NTILES // 2, BF, 1032], dt.float16,
                            kind="ExternalOutput")
    # bounce buffer, read-side-contiguous: [ring-half, m~ block, ring%12,
    # m~%16, re|im x bev] so each phase-2 read is one 96KB block
    gdram = nc.dram_tensor("gdram", [2, 8, 12, 16, 256], dt.float16)

    with tile.TileContext(nc) as tc:
        with (
            tc.tile_pool(name="xs", bufs=1) as xs_pool,
            tc.tile_pool(name="pws", bufs=1) as pw_pool,
            tc.tile_pool(name="g1", bufs=4) as g1_pool,
            tc.tile_pool(name="gs", bufs=8) as gs_pool,
            tc.tile_pool(name="os", bufs=3) as os_pool,
            tc.tile_pool(name="ps2", bufs=3, space="PSUM") as ps2,
            tc.tile_pool(name="ps1", bufs=2, space="PSUM") as ps1,
        ):
            QS = [nc.sync, nc.gpsimd, nc.scalar]

            # per-slot contiguous loads (slot-major layout): best measured
            # startup latency; fine-grained so ring s only waits on its slot
            # load and process slots smallest-first: the first MM then waits
            # on a 14KB transfer instead of a 155KB one (~20us less startup
            # stall), and the big loads stream behind the early compute
            xts = {}
            for s in reversed(range(SLOTS)):
                xt = xs_pool.tile([JP, 768], dt.float16, name=f"xt{s}",
                                  tag=f"xt{s}")
                QS[s % 3].dma_start(out=xt[0:r_s[s], :],
                                    in_=xef_d[s, 0:r_s[s], :])
                xts[s] = xt
            pw_sb = pw_pool.tile([SLOTS, PWCOLS], dt.float16)

            # ---- phase 1: 24 rings x 4 quadrant MMs ----
            for s in reversed(range(SLOTS)):
                K = r_s[s]
                xt = xts[s]
                g_ps = ps1.tile([128, 256], dt.float32, tag="g")
                nc.tensor.matmul(g_ps[0:64, 0:128], xt[0:K, 512:576],
                                 xt[0:K, 0:128])
                nc.tensor.matmul(g_ps[64:128, 0:128], xt[0:K, 576:640],
                                 xt[0:K, 128:256])
                nc.tensor.matmul(g_ps[0:64, 128:256], xt[0:K, 640:704],
                                 xt[0:K, 256:384])
                nc.tensor.matmul(g_ps[64:128, 128:256], xt[0:K, 704:768],
                                 xt[0:K, 384:512])
                g_sb = g1_pool.tile([128, 256], dt.float16, tag="ghl")
                # ACT and DVE evacuate half each, in parallel
                nc.scalar.copy(g_sb[:, 0:128], g_ps[:, 0:128])
                nc.vector.tensor_copy(g_sb[:, 128:256], g_ps[:, 128:256])
                # psum row m~ = (bp, m~%16); scatter to the 8 block slabs.
                # split writes gpsimd/sync: gpsimd alone was 2.4MB in ph1
                eng = nc.gpsimd if s % 2 == 0 else nc.sync
                eng.dma_start(out=gdram[s // 12, :, s % 12, :, :],
                              in_=g_sb[:])
                if s == 15:
                    # pw only gates phase 2; keep it off the startup path
                    nc.sync.dma_start(out=pw_sb[:, 0:4128],
                                      in_=pw_d[:, 0:4128])
                    nc.scalar.dma_start(out=pw_sb[:, 4128:PWCOLS],
                                        in_=pw_d[:, 4128:PWCOLS])

            # ---- phase 2: 64 m~ pairs, exact triangular ----
            o_sb = None
            for bp in range(4):
                glo = gs_pool.tile([SLOTS, 4096], dt.float16, tag="glo")
                ghi = gs_pool.tile([SLOTS, 4096], dt.float16, tag="ghi")
                for rr, (dst, src_b) in enumerate(((glo, bp), (ghi, 7 - bp))):
                    for rh in range(2):
                        eng = nc.sync if (rr + rh) % 2 == 0 else nc.scalar
                        eng.dma_start(
                            out=dst[12 * rh:12 * rh + 12, :],
                            in_=gdram[rh, src_b])
                for tt in range(8):
                    t = 8 * bp + tt
                    o_ps = ps2.tile([128, 2, 512], dt.float32, tag="o")
                    for b2 in range(2):
                        p = 2 * t + b2
                        a, _ = pairs[p]
                        i2 = a - 16 * bp
                        la = 128 - 2 * a
                        lb = 129 - la
                        po = 129 * p
                        nc.tensor.matmul(
                            o_ps[:, b2, 0:la],
                            glo[:, i2 * 256:i2 * 256 + 128],
                            pw_sb[:, po:po + la])
                        nc.tensor.matmul(
                            o_ps[:, b2, la:2 * la],
                            glo[:, i2 * 256 + 128:i2 * 256 + 256],
                            pw_sb[:, po:po + la])
                        nc.tensor.matmul(
                            o_ps[:, b2, 2 * la:2 * la + lb],
                            ghi[:, (15 - i2) * 256:(15 - i2) * 256 + 128],
                            pw_sb[:, po + la:po + 129])
                        nc.tensor.matmul(
                            o_ps[:, b2, 2 * la + lb:258],
                            ghi[:, (15 - i2) * 256 + 128:(15 - i2) * 256 + 256],
                            pw_sb[:, po + la:po + 129])
                    if t % 2 == 0:
                        o_sb = os_pool.tile([128, 1032], dt.float16, tag="ot")
                    oc = (t % 2) * 516
                    # ACT and DVE evacuate one psum bank each, in parallel
                    nc.scalar.copy(o_sb[:, oc:oc + 258], o_ps[:, 0, 0:258])
                    nc.vector.tensor_copy(o_sb[:, oc + 258:oc + 516],
                                          o_ps[:, 1, 0:258])
                    if t % 2 == 1:
                        QS[(t // 2) % 3].dma_start(
                            out=outp_d[t // 2], in_=o_sb[:])

    nc.compile()
    return nc


_CACHE = {}


def _get_compiled(r_s, pairs):
    if "nc" not in _CACHE:
        _CACHE["nc"] = _build_bass(r_s, pairs)
    return _CACHE["nc"]


def kernel(data, Pw, E_re, E_im, pad_idx):
    from concourse import bass_utils

    data = np.asarray(data)
    Pw = np.asarray(Pw, dtype=np.float32)
    E_re = np.asarray(E_re, dtype=np.float32)
    E_im = np.asarray(E_im, dtype=np.float32)

    nlon, order, r_s, pairs = _plan()
    offs = np.concatenate([[0], np.cumsum(nlon)[:-1]])
    # 'b e p v -> (b e v) p'
    x = np.ascontiguousarray(
        np.transpose(data, (0, 1, 3, 2)).reshape(BF, NPTS).astype(np.float32))

    in_maps = [
        _build_core_inputs(c, nlon, order, r_s, pairs, offs, x, E_re, E_im, Pw)
        for c in range(NCORES)
    ]

    nc = _get_compiled(r_s, pairs)
    res = bass_utils.run_bass_kernel_spmd(nc, in_maps, list(range(NCORES)))
    _CACHE["last_results"] = res

    total = np.zeros((NTILES // 2, BF, 1032), np.float64)
    for r in res.results:
        total += r["outp"].astype(np.float64)
    total = (total.reshape(NTILES // 2, BF, 2, 516)
             .transpose(0, 2, 1, 3).reshape(NTILES, BF, 516))

    coeffs = np.zeros((LMAX, MMAX, BF), np.complex128)
    for t in range(NTILES):
        for b2 in range(2):
            p = 2 * t + b2
            a, b = pairs[p]
            ma, mb = _true_m(a), _true_m(b)
            la, lb = 128 - ma, 128 - mb
            base = 258 * b2
            blk = total[t]
            re_a = blk[:, base:base + la]
            im_a = blk[:, base + la:base + 2 * la]
            re_b = blk[:, base + 2 * la:base + 2 * la + lb]
            im_b = blk[:, base + 2 * la + lb:base + 258]
            coeffs[ma:, ma, :] = (re_a + 1j * im_a).T
            coeffs[mb:, mb, :] = (re_b + 1j * im_b).T
    cc = coeffs.reshape(LMAX, MMAX, B, V)
    out = np.transpose(cc, (2, 0, 1, 3))[:, None]    # [b, 1, l, m, v]
    return out.astype(np.complex64)


# revision 33
# speedup vs baseline: 1.1583x; 1.1583x over previous
"""Octahedral SHT on 8 NeuronCores (Bass/Tile), quarter-folded fp16 design.

Strategy: shard the 192 latitude rings across 8 cores (24 each). The ragged
per-ring DFT is quarter-folded on the host using the cosine/sine symmetries
j<->n-j and j<->n/2-j: the folded x vectors (we/wo/ze/zo, one per m-parity x
re/im quadrant) have n/4+-1 rows, so every ring fits a single K<=101 matmul
chunk and the E matrix shrinks 4x. Tolerance is 2e-2, so all operands are
plain fp16 (measured pipeline error ~4.7e-4) - no hi/lo splitting.

Phase 1 (per ring): 4 MMs (re/im x even/odd m) -> psum [128 m~, 256 re|im],
rows 0:64 = even m, 64:128 = odd m. Evacuate f32->f16 (ACT/DVE alternate),
bounce through DRAM to transpose ring-index onto partitions.

Phase 2 (per m): out[bev, l] = G'[r, bev].T @ pw[r, l] with exact triangular
l-range (coeffs vanish for l < m). m's are processed in pairs (m~, 127-m~)
so each psum bank holds exactly 2*l(a)+2*l(b) = 258 f32 columns; one strided
evac per 2-bank tile, fp16 triangular output, host sums 8 partials.

DMA layouts are all contiguous DRAM blocks (>=96KB) with several transfers
in flight per queue: measured ~180GB/s aggregate vs ~20-90GB/s for strided
multi-row patterns. Rings are assigned to (core, slot) by sorted length
rank (slot s = rank 8s..8s+7, one per core) so the per-slot staircase row
count R_S[s] = 101-4s is SPMD-uniform and cuts zero-padding DMA ~45%.
"""
import numpy as np

NLAT, LMAX, MMAX = 192, 128, 128
B, V = 2, 64
BF = B * V            # 128 fused batch (b*64+v)
NCORES = 8
SLOTS = 24            # rings per core
JP = 104              # xef row-pad (>= max folded rows 101)
NPTS = 40320
PWCOLS = 129 * 64     # 8256: pair (a, 127-a) always has l_a + l_b = 129
NTILES = 32           # phase-2 psum tiles (2 m~ pairs each)


def _octa_nlon():
    half = NLAT // 2
    north = np.array([4 * (i + 1) + 16 for i in range(half)], dtype=np.int64)
    return np.concatenate([north, north[::-1]])


def _plan():
    nlon = _octa_nlon()
    order = np.argsort(-nlon, kind="stable")          # ring ids, length desc
    r_s = [int(nlon[order[8 * s]]) // 4 + 1 for s in range(SLOTS)]
    pairs = [(16 * bp + i, 127 - (16 * bp + i))
             for bp in range(4) for i in range(16)]   # m~ pairs, bp-major
    return nlon, order, r_s, pairs


def _true_m(mt):
    return 2 * mt if mt < 64 else 2 * (mt - 64) + 1


def _fold_ring(xr, n):
    """xr [BF, n] f32 -> (we, wo, ze, zo) with q+1, q, q-1, q rows (q=n//4)."""
    h, q = n // 2, n // 4
    u = np.empty((BF, h + 1), np.float32)
    u[:, 0] = xr[:, 0]
    u[:, h] = xr[:, h]
    u[:, 1:h] = xr[:, 1:h] + xr[:, :h:-1]
    v = xr[:, 1:h] - xr[:, :h:-1]                      # j=1..h-1 at col j-1
    jj = np.arange(1, q)
    we = np.empty((BF, q + 1), np.float32)
    we[:, 0] = u[:, 0] + u[:, h]
    we[:, q] = u[:, q]
    we[:, jj] = u[:, jj] + u[:, h - jj]
    wo = np.empty((BF, q), np.float32)
    wo[:, 0] = u[:, 0] - u[:, h]
    wo[:, jj] = u[:, jj] - u[:, h - jj]
    ze = v[:, jj - 1] - v[:, h - jj - 1]               # [BF, q-1]
    zo = np.empty((BF, q), np.float32)
    zo[:, jj - 1] = v[:, jj - 1] + v[:, h - jj - 1]
    zo[:, q - 1] = v[:, q - 1]
    return we, wo, ze, zo


def _build_core_inputs(c, nlon, order, r_s, pairs, offs, x, E_re, E_im, Pw):
    xef = np.zeros((SLOTS, JP, 768), np.float16)
    pw = np.zeros((SLOTS, PWCOLS), np.float16)
    for s in range(SLOTS):
        gid = int(order[8 * s + c])
        n = int(nlon[gid]); q = n // 4; o = int(offs[gid])
        we, wo, ze, zo = _fold_ring(x[:, o:o + n], n)
        xef[s, 0:q + 1, 0:128] = we.T
        xef[s, 0:q,     128:256] = wo.T
        xef[s, 0:q - 1, 256:384] = ze.T
        xef[s, 0:q,     384:512] = zo.T
        xef[s, 0:q + 1, 512:576] = E_re[gid, 0:q + 1, 0::2]
        xef[s, 0:q,     576:640] = E_re[gid, 0:q, 1::2]
        xef[s, 0:q - 1, 640:704] = E_im[gid, 1:q, 0::2]
        xef[s, 0:q,     704:768] = E_im[gid, 1:q + 1, 1::2]
        for p, (a, b) in enumerate(pairs):
            ma, mb = _true_m(a), _true_m(b)
            la = 128 - ma
            pw[s, 129 * p:129 * p + la] = Pw[ma:, ma, gid]
            pw[s, 129 * p + la:129 * (p + 1)] = Pw[mb:, mb, gid]
    return {"xef": xef, "pw": pw}


def _build_bass(r_s, pairs):
    import concourse.bass as bass
    import concourse.mybir as mybir
    from concourse import bacc, tile

    dt = mybir.dt
    nc = bacc.Bacc()

    xef_d = nc.dram_tensor("xef", [SLOTS, JP, 768], dt.float16,
                           kind="ExternalInput")
    pw_d = nc.dram_tensor("pw", [SLOTS, PWCOLS], dt.float16,
                          kind="ExternalInput")
    outp_d = nc.dram_tensor("outp", [NTILES // 2, BF, 1032], dt.float16,
                            kind="ExternalOutput")
    # bounce buffer, read-side-contiguous: [ring-half, m~ block, ring%12,
    # m~%16, re|im x bev] so each phase-2 read is one 96KB block
    gdram = nc.dram_tensor("gdram", [2, 8, 12, 16, 256], dt.float16)

    with tile.TileContext(nc) as tc:
        with (
            tc.tile_pool(name="xs", bufs=1) as xs_pool,
            tc.tile_pool(name="pws", bufs=1) as pw_pool,
            tc.tile_pool(name="g1", bufs=4) as g1_pool,
            tc.tile_pool(name="gs", bufs=8) as gs_pool,
            tc.tile_pool(name="os", bufs=3) as os_pool,
            tc.tile_pool(name="ps2", bufs=3, space="PSUM") as ps2,
            tc.tile_pool(name="ps1", bufs=2, space="PSUM") as ps1,
        ):
            QS = [nc.sync, nc.gpsimd, nc.scalar]

            # per-slot contiguous loads (slot-major layout): best measured
            # startup latency; fine-grained so ring s only waits on its slot
            # load and process slots smallest-first: the first MM then waits
            # on a 14KB transfer instead of a 155KB one (~20us less startup
            # stall), and the big loads stream behind the early compute
            xts = {}
            for s in reversed(range(SLOTS)):
                xt = xs_pool.tile([JP, 768], dt.float16, name=f"xt{s}",
                                  tag=f"xt{s}")
                QS[s % 3].dma_start(out=xt[0:r_s[s], :],
                                    in_=xef_d[s, 0:r_s[s], :])
                xts[s] = xt
            pw_sb = pw_pool.tile([SLOTS, PWCOLS], dt.float16)

            # ---- phase 1: 24 rings x 4 quadrant MMs ----
            for s in reversed(range(SLOTS)):
                K = r_s[s]
                xt = xts[s]
                g_ps = ps1.tile([128, 256], dt.float32, tag="g")
                nc.tensor.matmul(g_ps[0:64, 0:128], xt[0:K, 512:576],
                                 xt[0:K, 0:128])
                nc.tensor.matmul(g_ps[64:128, 0:128], xt[0:K, 576:640],
                                 xt[0:K, 128:256])
                nc.tensor.matmul(g_ps[0:64, 128:256], xt[0:K, 640:704],
                                 xt[0:K, 256:384])
                nc.tensor.matmul(g_ps[64:128, 128:256], xt[0:K, 704:768],
                                 xt[0:K, 384:512])
                g_sb = g1_pool.tile([128, 256], dt.float16, tag="ghl")
                # ACT and DVE evacuate half each, in parallel
                nc.scalar.copy(g_sb[:, 0:128], g_ps[:, 0:128])
                nc.vector.tensor_copy(g_sb[:, 128:256], g_ps[:, 128:256])
                # psum row m~ = (bp, m~%16); scatter to the 8 block slabs.
                # SWDGE (gpsimd) issues cheaply and its sems don't straggle
                nc.gpsimd.dma_start(out=gdram[s // 12, :, s % 12, :, :],
                                    in_=g_sb[:])
                if s == 15:
                    # pw only gates phase 2; keep it off the startup path
                    nc.sync.dma_start(out=pw_sb[:, 0:4128],
                                      in_=pw_d[:, 0:4128])
                    nc.scalar.dma_start(out=pw_sb[:, 4128:PWCOLS],
                                        in_=pw_d[:, 4128:PWCOLS])

            # ---- phase 2: 64 m~ pairs, exact triangular ----
            o_sb = None
            for bp in range(4):
                glo = gs_pool.tile([SLOTS, 4096], dt.float16, tag="glo")
                ghi = gs_pool.tile([SLOTS, 4096], dt.float16, tag="ghi")
                for rr, (dst, src_b) in enumerate(((glo, bp), (ghi, 7 - bp))):
                    for rh in range(2):
                        eng = nc.sync if (rr + rh) % 2 == 0 else nc.scalar
                        eng.dma_start(
                            out=dst[12 * rh:12 * rh + 12, :],
                            in_=gdram[rh, src_b])
                for tt in range(8):
                    t = 8 * bp + tt
                    o_ps = ps2.tile([128, 2, 512], dt.float32, tag="o")
                    for b2 in range(2):
                        p = 2 * t + b2
                        a, _ = pairs[p]
                        i2 = a - 16 * bp
                        la = 128 - 2 * a
                        lb = 129 - la
                        po = 129 * p
                        nc.tensor.matmul(
                            o_ps[:, b2, 0:la],
                            glo[:, i2 * 256:i2 * 256 + 128],
                            pw_sb[:, po:po + la])
                        nc.tensor.matmul(
                            o_ps[:, b2, la:2 * la],
                            glo[:, i2 * 256 + 128:i2 * 256 + 256],
                            pw_sb[:, po:po + la])
                        nc.tensor.matmul(
                            o_ps[:, b2, 2 * la:2 * la + lb],
                            ghi[:, (15 - i2) * 256:(15 - i2) * 256 + 128],
                            pw_sb[:, po + la:po + 129])
                        nc.tensor.matmul(
                            o_ps[:, b2, 2 * la + lb:258],
                            ghi[:, (15 - i2) * 256 + 128:(15 - i2) * 256 + 256],
                            pw_sb[:, po + la:po + 129])
                    if t % 2 == 0:
                        o_sb = os_pool.tile([128, 1032], dt.float16, tag="ot")
                    oc = (t % 2) * 516
                    # ACT and DVE evacuate one psum bank each, in parallel
                    nc.scalar.copy(o_sb[:, oc:oc + 258], o_ps[:, 0, 0:258])
                    nc.vector.tensor_copy(o_sb[:, oc + 258:oc + 516],
                                          o_ps[:, 1, 0:258])
                    if t % 2 == 1:
                        QS[(t // 2) % 3].dma_start(
                            out=outp_d[t // 2], in_=o_sb[:])

    nc.compile()
    return nc


_CACHE = {}


def _get_compiled(r_s, pairs):
    if "nc" not in _CACHE:
        _CACHE["nc"] = _build_bass(r_s, pairs)
    return _CACHE["nc"]


def kernel(data, Pw, E_re, E_im, pad_idx):
    from concourse import bass_utils

    data = np.asarray(data)
    Pw = np.asarray(Pw, dtype=np.float32)
    E_re = np.asarray(E_re, dtype=np.float32)
    E_im = np.asarray(E_im, dtype=np.float32)

    nlon, order, r_s, pairs = _plan()
    offs = np.concatenate([[0], np.cumsum(nlon)[:-1]])
    # 'b e p v -> (b e v) p'
    x = np.ascontiguousarray(
        np.transpose(data, (0, 1, 3, 2)).reshape(BF, NPTS).astype(np.float32))

    in_maps = [
        _build_core_inputs(c, nlon, order, r_s, pairs, offs, x, E_re, E_im, Pw)
        for c in range(NCORES)
    ]

    nc = _get_compiled(r_s, pairs)
    res = bass_utils.run_bass_kernel_spmd(nc, in_maps, list(range(NCORES)))
    _CACHE["last_results"] = res

    total = np.zeros((NTILES // 2, BF, 1032), np.float64)
    for r in res.results:
        total += r["outp"].astype(np.float64)
    total = (total.reshape(NTILES // 2, BF, 2, 516)
             .transpose(0, 2, 1, 3).reshape(NTILES, BF, 516))

    coeffs = np.zeros((LMAX, MMAX, BF), np.complex128)
    for t in range(NTILES):
        for b2 in range(2):
            p = 2 * t + b2
            a, b = pairs[p]
            ma, mb = _true_m(a), _true_m(b)
            la, lb = 128 - ma, 128 - mb
            base = 258 * b2
            blk = total[t]
            re_a = blk[:, base:base + la]
            im_a = blk[:, base + la:base + 2 * la]
            re_b = blk[:, base + 2 * la:base + 2 * la + lb]
            im_b = blk[:, base + 2 * la + lb:base + 258]
            coeffs[ma:, ma, :] = (re_a + 1j * im_a).T
            coeffs[mb:, mb, :] = (re_b + 1j * im_b).T
    cc = coeffs.reshape(LMAX, MMAX, B, V)
    out = np.transpose(cc, (2, 0, 1, 3))[:, None]    # [b, 1, l, m, v]
    return out.astype(np.complex64)
